# revision 10
# baseline (speedup 1.0000x reference)
"""GAT message-passing kernel for TRN2 (8 NeuronCores, SPMD).

Algorithm (matches the jax reference up to a softmax shift, which cancels):
  proj = src @ W_src.T ; s_src[n,h] = src[n].w_s[h] ; s_trg[n,h] = trg[n].w_t[h]
  score_e = leakyrelu(s_src[si]+s_trg[ti]) ; p_e = exp(score_e - C_OFF)
  out[t,h,:] = sum_{e: ti=t} p_e * proj[si_e,h,:] / (sum p_e + eps)

Sharding: edges sorted by target; core c owns targets [c*TPC,(c+1)*TPC).
Windows of 128 targets are processed in superwindow pairs; per (window,slab)
bucket the edges are sorted by local target so the edge->target one-hot is a
staircase built from per-target [start,end) run bounds (clamp + is_equal
against an iota), with no replicated-tl DRAM traffic. Per-edge rows
(proj bf16 | s_src f32) come from a packed 512B-row node table (built on
device in P0, only 288B written) via dma_gather merged across the
superwindow (int16 idx, 32768-row slabs). s_trg stays in SBUF (P0b).
"""
import os
import numpy as np
import ml_dtypes

import concourse.bacc as bacc
import concourse.mybir as mybir
import concourse.tile as tile
from concourse.bass_utils import run_bass_kernel_spmd

BF16 = mybir.dt.bfloat16
F32 = mybir.dt.float32
I16 = mybir.dt.int16

NH, FOUT, D = 8, 16, 128
HF = NH * FOUT  # 128
NEG_SLOPE = 0.2
C_OFF = 16.0
SLAB = 32768
ROW = 256          # bf16 slots per node-table row (512B); 144 written
RWR = 144          # written row prefix (proj 128 | s_src f32 as 16 bf16)
SW = 2             # windows per superwindow
MAX_CT = 8         # max tiles (x128 idx) per dma_gather call
NQ = 4             # SWDGE queues
EMPTY_START = 60000.0

LAST_EXEC_NS = None


def _install_trace_shim():
    """Register the axon NTFF profile hook (missing antenv.axon_hooks shim)."""
    import sys
    import types

    if "antenv.axon_hooks" in sys.modules:
        return True
    try:
        mod = types.ModuleType("antenv.axon_hooks")
        mod._hook = None
        mod.set_axon_ntff_profile_hook = lambda h: setattr(mod, "_hook", h)
        mod.get_axon_ntff_profile_hook = lambda: mod._hook
        from trn_agent_boot.trn_boot import _ntff_profile_via_ctypes

        mod._hook = _ntff_profile_via_ctypes("/opt/axon/libaxon_pjrt.so")
        sys.modules["antenv.axon_hooks"] = mod
        import concourse.bass_utils as bu

        bu.upload_artifacts = lambda tmpdir: tmpdir
        return True
    except Exception:
        return False


def _wrap_idx(v):
    """[ct*128] int array -> [128, ct*8] int16 wrapped+replicated layout."""
    w = np.asarray(v, dtype=np.int16).reshape(-1, 16).T  # [16, ct*8]
    return np.tile(w, (8, 1))


def build_schedule(si, ti, N, ncores):
    """Shared SPMD schedule (bucket/tile emission order) + per-core data.

    Emission: superwindows of SW windows; within a superwindow, slabs-major
    so one gather call covers all member windows' tiles for that slab.
    Within each (w, s) bucket edges are sorted by local target (ctl).
    """
    TPC = N // ncores
    WPC = (TPC + 127) // 128
    NPAD = ((N + 127) // 128) * 128
    nslabs = (NPAD + SLAB - 1) // SLAB
    NSW = (WPC + SW - 1) // SW

    si = np.asarray(si, dtype=np.int64)
    ti = np.asarray(ti, dtype=np.int64)
    core = ti // TPC
    tloc = ti - core * TPC
    w_of = tloc >> 7
    s_of = si >> 15
    ctl = tloc & 127

    counts = np.zeros((ncores, WPC, nslabs), dtype=np.int64)
    np.add.at(counts, (core, w_of, s_of), 1)
    NT = np.ceil(counts.max(axis=0) / 128).astype(np.int64)  # [WPC, nslabs]

    # --- shared emission structure ---
    buckets = []            # (w, s, t0b, nt) global tile offset + length
    tiles_w = [[] for _ in range(WPC)]   # global tile ids per window
    gcalls = []             # per g: list of (s, t0c, ct)
    sw_ranges = []          # per g: (t0, Tg)
    sw_windows = []         # per g: member window ids
    t = 0
    for g in range(NSW):
        ws = [g * SW + i for i in range(SW) if g * SW + i < WPC]
        sw_windows.append(ws)
        t0 = t
        calls = []
        for s in range(nslabs):
            c0 = t
            for w in ws:
                nt = int(NT[w, s])
                if nt == 0:
                    continue
                buckets.append((w, s, t, nt))
                tiles_w[w].extend(range(t, t + nt))
                t += nt
            k = c0
            while k < t:
                ct = min(MAX_CT, t - k)
                calls.append((s, k, ct))
                k += ct
        gcalls.append(calls)
        sw_ranges.append((t0, t - t0))
    T_total = t
    NB = max(len(buckets), 1)
    TGMAX = max((r[1] for r in sw_ranges), default=1)
    REGMAX = int(max([b[3] for b in buckets], default=1)) * 128

    # --- per-core data ---
    per_core = []
    for c in range(ncores):
        m = core == c
        csi = si[m]
        cw, cs, cctl = w_of[m], s_of[m], ctl[m]
        order = np.lexsort((cctl, cs, cw))
        csi, cw, cs, cctl = (a[order] for a in (csi, cw, cs, cctl))
        keys = cw * nslabs + cs
        starts = np.searchsorted(keys, np.arange(WPC * nslabs))
        ends = np.searchsorted(keys, np.arange(WPC * nslabs), side="right")

        si_loc = np.zeros(T_total * 128, dtype=np.int64)
        tl = np.full(T_total * 128, 255, dtype=np.int64)
        startc = np.full((128, NB), EMPTY_START, dtype=np.float32)
        endc = np.full((128, NB), -1.0, dtype=np.float32)
        p128 = np.arange(128)
        for bi, (w, s, t0b, nt) in enumerate(buckets):
            a, b = starts[w * nslabs + s], ends[w * nslabs + s]
            cnt = b - a
            if cnt == 0:
                continue
            slot0 = t0b * 128
            si_loc[slot0 : slot0 + cnt] = csi[a:b] - s * SLAB
            seg = cctl[a:b]
            tl[slot0 : slot0 + cnt] = seg
            st = np.searchsorted(seg, p128, side="left").astype(np.float32)
            en = np.searchsorted(seg, p128, side="right").astype(np.float32)
            nonz = en > st
            startc[nonz, bi] = st[nonz]
            endc[nonz, bi] = en[nonz] - 1.0

        blocks = []
        for g in range(NSW):
            for (s, t0c, ct) in gcalls[g]:
                blocks.append(_wrap_idx(si_loc[t0c * 128 : (t0c + ct) * 128]))
        idx1 = np.concatenate(blocks, axis=1) if blocks else np.zeros((128, 8), np.int16)
        tl_bf = tl.reshape(T_total, 128).T.astype(np.float32).astype(ml_dtypes.bfloat16)
        per_core.append((idx1, tl_bf, startc, endc))

    sched = dict(TPC=TPC, WPC=WPC, NPAD=NPAD, nslabs=nslabs, NSW=NSW,
                 T_total=T_total, NB=NB, TGMAX=TGMAX, REGMAX=REGMAX,
                 buckets=buckets, tiles_w=tiles_w, gcalls=gcalls,
                 sw_ranges=sw_ranges, sw_windows=sw_windows)
    return sched, per_core


def build_nc(N, sched):
    WPC, NPAD, NSW = sched["WPC"], sched["NPAD"], sched["NSW"]
    T_total, NB = sched["T_total"], sched["NB"]
    TGMAX, REGMAX = sched["TGMAX"], sched["REGMAX"]
    buckets, tiles_w = sched["buckets"], sched["tiles_w"]
    gcalls, sw_ranges, sw_windows = sched["gcalls"], sched["sw_ranges"], sched["sw_windows"]
    NT0 = NPAD // 128
    LROWS = WPC * 128

    # window id per global tile
    wof = np.zeros(T_total, dtype=np.int64)
    for w, tls in enumerate(tiles_w):
        for k in tls:
            wof[k] = w

    nc = bacc.Bacc("TRN2", target_bir_lowering=False, num_swdge_queues=NQ)
    srcT = nc.declare_dram_parameter("srcT", [128, NPAD], BF16, isOutput=False)
    trgTl = nc.declare_dram_parameter("trgTl", [128, LROWS], BF16, isOutput=False)
    wext = nc.declare_dram_parameter("wext", [128, 144], BF16, isOutput=False)
    iota = nc.declare_dram_parameter("iota", [128, 128], BF16, isOutput=False)
    iotaf = nc.declare_dram_parameter("iotaf", [128, REGMAX], F32, isOutput=False)
    idx1 = nc.declare_dram_parameter("idx1", [128, max(T_total * 8, 8)], I16, isOutput=False)
    tlp = nc.declare_dram_parameter("tl", [128, max(T_total, 1)], BF16, isOutput=False)
    startp = nc.declare_dram_parameter("startc", [128, NB], F32, isOutput=False)
    endp = nc.declare_dram_parameter("endc", [128, NB], F32, isOutput=False)
    outp = nc.declare_dram_parameter("out", [NSW * SW * 128, HF], F32, isOutput=True)
    table = nc.dram_tensor("table", [NPAD, ROW], BF16)

    qrr = [0]

    def next_q():
        q = qrr[0]
        qrr[0] = (q + 1) % NQ
        return q

    with tile.TileContext(nc) as tc:
        with tc.tile_pool(name="const", bufs=1) as cp:
            wext_sb = cp.tile([128, 144], BF16)
            nc.sync.dma_start(out=wext_sb[:], in_=wext[:, :])
            iota_sb = cp.tile([128, 128], BF16)
            nc.sync.dma_start(out=iota_sb[:], in_=iota[:, :])
            iotaf_sb = cp.tile([128, REGMAX], F32)
            nc.sync.dma_start(out=iotaf_sb[:], in_=iotaf[:, :])
            tl_sb = cp.tile([128, max(T_total, 1)], BF16)
            nc.sync.dma_start(out=tl_sb[:], in_=tlp[:, :])
            idx1_sb = cp.tile([128, max(T_total * 8, 8)], I16)
            nc.sync.dma_start(out=idx1_sb[:], in_=idx1[:, :])
            start_sb = cp.tile([128, NB], F32)
            nc.sync.dma_start(out=start_sb[:], in_=startp[:, :])
            end_sb = cp.tile([128, NB], F32)
            nc.sync.dma_start(out=end_sb[:], in_=endp[:, :])
            cbias = cp.tile([128, 1], F32)
            nc.vector.memset(cbias[:], -C_OFF)
            swall = cp.tile([128, WPC, 16], BF16)  # s_trg hi|lo per local target

            # ---- P0b: core-local s_trg into SBUF ----
            with (
                tc.tile_pool(name="pbin", bufs=2) as pbin,
                tc.tile_pool(name="pbtmp", bufs=2) as pbtmp,
                tc.tile_pool(name="pbps", bufs=2, space="PSUM") as pbps,
            ):
                j = 0
                while j < WPC:
                    g = min(3, WPC - j)
                    t_t = pbin.tile([128, 128 * 3], BF16, tag="t", name=f"t{j}")
                    nc.sync.dma_start(out=t_t[:, : 128 * g], in_=trgTl[:, j * 128 : (j + g) * 128])
                    psB = pbps.tile([128, 8 * 3], F32, tag="psB", name=f"pb{j}")
                    for k in range(g):
                        nc.tensor.matmul(
                            out=psB[:, k * 8 : (k + 1) * 8],
                            lhsT=t_t[:, k * 128 : (k + 1) * 128],
                            rhs=wext_sb[:, 136:144],
                            start=True, stop=True,
                        )
                    psB_r = psB[:].rearrange("p (k c) -> p k c", c=8)
                    nc.vector.tensor_copy(out=swall[:, j : j + g, 0:8], in_=psB_r[:, :g, :])
                    lob = pbtmp.tile([128, 3, 8], F32, tag="lob", name=f"lo{j}")
                    nc.vector.tensor_tensor(
                        out=lob[:, :g, :], in0=psB_r[:, :g, :], in1=swall[:, j : j + g, 0:8],
                        op=mybir.AluOpType.subtract,
                    )
                    nc.scalar.copy(out=swall[:, j : j + g, 8:16], in_=lob[:, :g, :])
                    j += g

            # ---- P0: packed node table (proj bf16 | s_src f32), 288B rows ----
            CH = 12
            with (
                tc.tile_pool(name="p0in", bufs=2) as p0in,
                tc.tile_pool(name="p0row", bufs=2) as p0row,
                tc.tile_pool(name="p0ps", bufs=4, space="PSUM") as p0ps,
            ):
                j = 0
                while j < NT0:
                    ch = min(CH, NT0 - j)
                    j0 = j * 128
                    s_t = p0in.tile([128, 128 * CH], BF16, tag="s", name=f"s{j}")
                    nc.sync.dma_start(out=s_t[:, : 128 * ch], in_=srcT[:, j0 : j0 + 128 * ch])
                    row = p0row.tile([128, CH, RWR], BF16, tag="row", name=f"r{j}")
                    row_f32 = row[:].bitcast(F32)  # [128, CH, 72]
                    k0 = 0
                    while k0 < ch:
                        g = min(3, ch - k0)
                        psA = p0ps.tile([128, 136 * 3], F32, tag="psA", name=f"pa{j}_{k0}")
                        for k in range(g):
                            nc.tensor.matmul(
                                out=psA[:, k * 136 : (k + 1) * 136],
                                lhsT=s_t[:, (k0 + k) * 128 : (k0 + k + 1) * 128],
                                rhs=wext_sb[:, 0:136],
                                start=True, stop=True,
                            )
                        psA_r = psA[:].rearrange("p (k c) -> p k c", c=136)
                        nc.scalar.copy(out=row[:, k0 : k0 + g, 0:HF], in_=psA_r[:, :g, 0:HF])
                        nc.scalar.copy(
                            out=row_f32[:, k0 : k0 + g, 64:72], in_=psA_r[:, :g, 128:136]
                        )
                        k0 += g
                    nc.sync.dma_start(
                        out=table[j0 : j0 + 128 * ch, 0:RWR].rearrange(
                            "(k p) c -> p k c", p=128
                        ),
                        in_=row[:, :ch, :],
                    )
                    j += ch

            # ---- P1: edge pass, one superwindow (SW windows) at a time ----
            with (
                tc.tile_pool(name="g1p", bufs=2) as g1p,
                tc.tile_pool(name="ohtp", bufs=2) as ohtp,
                tc.tile_pool(name="zp", bufs=2) as zp,
                tc.tile_pool(name="psep", bufs=2, space="PSUM") as psep,
                tc.tile_pool(name="scp", bufs=2) as scp,
                tc.tile_pool(name="whp", bufs=2) as whp,
                tc.tile_pool(name="ohp", bufs=2) as ohp,
                tc.tile_pool(name="pswp", bufs=4, space="PSUM") as pswp,
                tc.tile_pool(name="epi", bufs=2) as epi,
            ):
                c1off = 0
                for g in range(NSW):
                    t0, Tg = sw_ranges[g]
                    ws = sw_windows[g]
                    outt = epi.tile([128, SW, HF], F32, tag="outt", name=f"o{g}")
                    if Tg > 0:
                        G1 = g1p.tile([128, TGMAX, ROW], BF16, tag="g1", name=f"g1_{g}")
                        for (s, t0c, ct) in gcalls[g]:
                            sb = s * SLAB
                            se = min(sb + SLAB, NPAD)
                            nc.gpsimd.dma_gather(
                                G1[:, t0c - t0 : t0c - t0 + ct, :],
                                table[sb:se, :],
                                idx1_sb[:, c1off : c1off + ct * 8],
                                ct * 128, ct * 128, ROW,
                                queue_num=next_q(),
                            )
                            c1off += ct * 8
                        # edge->target one-hot (transposed): staircase from
                        # per-target run bounds over tl-sorted buckets.
                        ohT = ohtp.tile([128, TGMAX * 128], BF16, tag="ohT", name=f"oT{g}")
                        for bi, (w, s, t0b, nt) in enumerate(buckets):
                            if not (t0 <= t0b < t0 + Tg):
                                continue
                            reg = nt * 128
                            kb = (t0b - t0) * 128
                            z = zp.tile([128, REGMAX], F32, tag="z", name=f"z{g}_{bi}")
                            nc.vector.tensor_scalar(
                                out=z[:, :reg], in0=iotaf_sb[:, :reg],
                                scalar1=end_sb[:, bi : bi + 1],
                                scalar2=start_sb[:, bi : bi + 1],
                                op0=mybir.AluOpType.min, op1=mybir.AluOpType.max,
                            )
                            nc.vector.tensor_tensor(
                                out=ohT[:, kb : kb + reg], in0=z[:, :reg],
                                in1=iotaf_sb[:, :reg], op=mybir.AluOpType.is_equal,
                            )
                        pse = psep.tile([128, TGMAX, 16], F32, tag="pse", name=f"pe{g}")
                        for kk in range(t0, t0 + Tg):
                            nc.tensor.matmul(
                                out=pse[:, kk - t0, :],
                                lhsT=ohT[:, (kk - t0) * 128 : (kk - t0 + 1) * 128],
                                rhs=swall[:, int(wof[kk]), :],
                                start=True, stop=True,
                            )
                        G1f = G1[:].bitcast(F32)  # [128, TGMAX, 128]
                        sc0 = scp.tile([128, TGMAX, 8], F32, tag="sc0", name=f"s0_{g}")
                        nc.vector.tensor_tensor(
                            out=sc0[:, :Tg, :], in0=G1f[:, :Tg, 64:72],
                            in1=pse[:, :Tg, 0:8], op=mybir.AluOpType.add,
                        )
                        sc = scp.tile([128, TGMAX, 8], F32, tag="sc", name=f"sc{g}")
                        nc.vector.tensor_tensor(
                            out=sc[:, :Tg, :], in0=sc0[:, :Tg, :],
                            in1=pse[:, :Tg, 8:16], op=mybir.AluOpType.add,
                        )
                        # exp(leakyrelu(s) - C) = max(exp(s - C), exp(0.2*s - C))
                        e1 = scp.tile([128, TGMAX, 8], F32, tag="e1", name=f"e1_{g}")
                        nc.scalar.activation(
                            e1[:, :Tg, :], sc[:, :Tg, :],
                            mybir.ActivationFunctionType.Exp, bias=cbias[:, 0:1],
                        )
                        e2 = scp.tile([128, TGMAX, 8], F32, tag="e2", name=f"e2_{g}")
                        nc.scalar.activation(
                            e2[:, :Tg, :], sc[:, :Tg, :],
                            mybir.ActivationFunctionType.Exp, bias=cbias[:, 0:1],
                            scale=NEG_SLOPE,
                        )
                        e_sb = scp.tile([128, TGMAX, 8], BF16, tag="e", name=f"e{g}")
                        nc.vector.tensor_tensor(
                            out=e_sb[:, :Tg, :], in0=e1[:, :Tg, :],
                            in1=e2[:, :Tg, :], op=mybir.AluOpType.max,
                        )
                        wt = whp.tile([128, TGMAX, 136], BF16, tag="wt", name=f"wt{g}")
                        nc.scalar.copy(out=wt[:, :Tg, 128:136], in_=e_sb[:, :Tg, :])
                        e_b = e_sb[:, :Tg, :].rearrange(
                            "p w (h o) -> p w h o", o=1
                        ).to_broadcast([128, Tg, 8, 16])
                        nc.vector.tensor_tensor(
                            out=wt[:, :Tg, 0:128].rearrange("p w (h f) -> p w h f", f=16),
                            in0=G1[:, :Tg, 0:128].rearrange("p w (h f) -> p w h f", f=16),
                            in1=e_b, op=mybir.AluOpType.mult,
                        )
                        oh = ohp.tile([128, TGMAX, 128], BF16, tag="oh", name=f"oh{g}")
                        iota_b = iota_sb[:].rearrange("p (o c) -> p o c", o=1).to_broadcast(
                            [128, Tg, 128]
                        )
                        tl_b = tl_sb[:, t0 : t0 + Tg].rearrange(
                            "p (w o) -> p w o", o=1
                        ).to_broadcast([128, Tg, 128])
                        nc.vector.tensor_tensor(
                            out=oh[:, :Tg, :], in0=iota_b, in1=tl_b,
                            op=mybir.AluOpType.is_equal,
                        )
                    for wi, w in enumerate(ws):
                        tls = tiles_w[w]
                        if not tls:
                            nc.vector.memset(outt[:, wi, :], 0.0)
                            continue
                        psw = pswp.tile([128, 136], F32, tag="ps", name=f"ps{g}_{wi}")
                        for i, kk in enumerate(tls):
                            nc.tensor.matmul(
                                out=psw[:], lhsT=oh[:, kk - t0, :], rhs=wt[:, kk - t0, :],
                                start=(i == 0), stop=(i == len(tls) - 1),
                            )
                        dn = epi.tile([128, 8], F32, tag="dn", name=f"dn{g}_{wi}")
                        nc.vector.tensor_scalar_add(out=dn[:], in0=psw[:, 128:136], scalar1=1e-16)
                        rc = epi.tile([128, 8], F32, tag="rc", name=f"rc{g}_{wi}")
                        nc.vector.reciprocal(out=rc[:], in_=dn[:])
                        rc_b = rc[:].rearrange("p (h o) -> p h o", o=1).to_broadcast([128, 8, 16])
                        nc.vector.tensor_tensor(
                            out=outt[:, wi, :].rearrange("p (h f) -> p h f", f=16),
                            in0=psw[:, 0:HF].rearrange("p (h f) -> p h f", f=16),
                            in1=rc_b, op=mybir.AluOpType.mult,
                        )
                    nc.sync.dma_start(
                        out=outp[g * SW * 128 : (g * SW + len(ws)) * 128, :].rearrange(
                            "(k p) c -> p k c", p=128
                        ),
                        in_=outt[:, : len(ws), :],
                    )
    nc.compile()
    return nc


def host_prep(trg, src, W_trg, W_src, a_src, a_trg, N, ncores, TPC, WPC, NPAD, REGMAX):
    LROWS = WPC * 128
    src2 = np.asarray(src, dtype=np.float32).reshape(-1, D)[:N]
    trg2 = np.asarray(trg, dtype=np.float32).reshape(-1, D)[:N]
    W_src = np.asarray(W_src, dtype=np.float32)
    W_trg = np.asarray(W_trg, dtype=np.float32)
    a_src = np.asarray(a_src, dtype=np.float32)
    a_trg = np.asarray(a_trg, dtype=np.float32)
    w_s = np.einsum("hf,hfd->hd", a_src, W_src.reshape(NH, FOUT, D))
    w_t = np.einsum("hf,hfd->hd", a_trg, W_trg.reshape(NH, FOUT, D))
    wext = np.zeros((128, 144), dtype=np.float32)
    wext[:, 0:HF] = W_src.T
    wext[:, HF : HF + 8] = w_s.T
    wext[:, 136:144] = w_t.T
    bf = ml_dtypes.bfloat16
    srcT = np.zeros((128, NPAD), dtype=np.float32)
    srcT[:, :N] = src2.T
    trgTls = []
    for c in range(ncores):
        t = np.zeros((128, LROWS), dtype=np.float32)
        t[:, :TPC] = trg2[c * TPC : (c + 1) * TPC].T
        trgTls.append(t.astype(bf))
    iota = np.tile(np.arange(128, dtype=np.float32), (128, 1))
    iotaf = np.tile(np.arange(REGMAX, dtype=np.float32), (128, 1))
    return srcT.astype(bf), trgTls, wext.astype(bf), iota.astype(bf), iotaf


_CACHE = {}


def run_graph(trg, src, edge_index, W_trg, W_src, a_src, a_trg, N, ncores,
              trace=False):
    global LAST_EXEC_NS
    si = np.asarray(edge_index[0], dtype=np.int64)
    ti = np.asarray(edge_index[1], dtype=np.int64)
    sched, per_core = build_schedule(si, ti, N, ncores)
    TPC, WPC, NPAD = sched["TPC"], sched["WPC"], sched["NPAD"]
    T_total, REGMAX = sched["T_total"], sched["REGMAX"]

    srcT, trgTls, wext, iota, iotaf = host_prep(
        trg, src, W_trg, W_src, a_src, a_trg, N, ncores, TPC, WPC, NPAD, REGMAX
    )

    key = (N, ncores, T_total, tuple(int(b[2]) for b in sched["buckets"]))
    if key not in _CACHE:
        _CACHE[key] = build_nc(N, sched)
    nc = _CACHE[key]

    in_maps = []
    for c in range(ncores):
        idx1, tl_bf, startc, endc = per_core[c]
        i1 = np.zeros((128, max(T_total * 8, 8)), dtype=np.int16)
        i1[:, : idx1.shape[1]] = idx1
        tlz = np.full((128, max(T_total, 1)), 255.0, dtype=ml_dtypes.bfloat16)
        tlz[:, : tl_bf.shape[1]] = tl_bf
        in_maps.append(
            {"srcT": srcT, "trgTl": trgTls[c], "wext": wext, "iota": iota,
             "iotaf": iotaf, "idx1": i1, "tl": tlz,
             "startc": startc, "endc": endc}
        )

    if trace:
        trace = _install_trace_shim()
    res = run_bass_kernel_spmd(nc, in_maps, core_ids=list(range(ncores)), trace=trace)
    LAST_EXEC_NS = res.exec_time_ns
    out = np.zeros((N, HF), dtype=np.float32)
    for c in range(ncores):
        out[c * TPC : (c + 1) * TPC] = res.results[c]["out"][:TPC]
    return out


def kernel(trg, src, edge_index, W_trg, W_src, a_src, a_trg):
    N = 100000
    out = run_graph(trg, src, edge_index, W_trg, W_src, a_src, a_trg, N, 8,
                    trace=bool(os.environ.get("KERNEL_TRACE")))
    return out.reshape(1, N, HF)


# revision 19
# speedup vs baseline: 1.4483x; 1.4483x over previous
"""GAT message-passing kernel for TRN2 (8 NeuronCores, SPMD).

Algorithm (matches the jax reference up to a softmax shift, which cancels):
  proj = src @ W_src.T ; s_src[n,h] = src[n].w_s[h] ; s_trg[n,h] = trg[n].w_t[h]
  score_e = leakyrelu(s_src[si]+s_trg[ti]) ; p_e = exp(score_e - C_OFF)
  out[t,h,:] = sum_{e: ti=t} p_e * proj[si_e,h,:] / (sum p_e + eps)

Sharding: edges sorted by target; core c owns targets [c*TPC,(c+1)*TPC).
Windows of 128 targets are processed in superwindow pairs; per (window,slab)
bucket the edges are sorted by local target so the edge->target one-hot is a
staircase built from per-target [start,end) run bounds (clamp + is_equal
against an iota), with no replicated-tl DRAM traffic. Per-edge rows
(proj bf16 | s_src f32) come from a packed 512B-row node table (built on
device in P0, only 288B written) via dma_gather merged across the
superwindow (int16 idx, 32768-row slabs). s_trg stays in SBUF (P0b).
"""
import os
import numpy as np
import ml_dtypes

import concourse.bacc as bacc
import concourse.mybir as mybir
import concourse.tile as tile
from concourse.bass_utils import run_bass_kernel_spmd

BF16 = mybir.dt.bfloat16
F32 = mybir.dt.float32
I16 = mybir.dt.int16

NH, FOUT, D = 8, 16, 128
HF = NH * FOUT  # 128
NEG_SLOPE = 0.2
C_OFF = 16.0
SLAB = 32768
ROW = 256          # bf16 slots per node-table row (512B); 144 written
RWR = 144          # written row prefix (proj 128 | s_src f32 as 16 bf16)
SW = 2             # windows per superwindow
MAX_CT = 8         # max tiles (x128 idx) per dma_gather call
NQ = 4             # SWDGE queues
EMPTY_START = 60000.0

LAST_EXEC_NS = None


def _install_trace_shim():
    """Register the axon NTFF profile hook (missing antenv.axon_hooks shim)."""
    import sys
    import types

    if "antenv.axon_hooks" in sys.modules:
        return True
    try:
        mod = types.ModuleType("antenv.axon_hooks")
        mod._hook = None
        mod.set_axon_ntff_profile_hook = lambda h: setattr(mod, "_hook", h)
        mod.get_axon_ntff_profile_hook = lambda: mod._hook
        from trn_agent_boot.trn_boot import _ntff_profile_via_ctypes

        mod._hook = _ntff_profile_via_ctypes("/opt/axon/libaxon_pjrt.so")
        sys.modules["antenv.axon_hooks"] = mod
        import concourse.bass_utils as bu

        bu.upload_artifacts = lambda tmpdir: tmpdir
        return True
    except Exception:
        return False


def _wrap_idx(v):
    """[ct*128] int array -> [128, ct*8] int16 wrapped+replicated layout."""
    w = np.asarray(v, dtype=np.int16).reshape(-1, 16).T  # [16, ct*8]
    return np.tile(w, (8, 1))


def build_schedule(si, ti, N, ncores):
    """Shared SPMD schedule (bucket/tile emission order) + per-core data.

    Emission: superwindows of SW windows; within a superwindow, slabs-major
    so one gather call covers all member windows' tiles for that slab.
    Within each (w, s) bucket edges are sorted by local target (ctl).
    """
    TPC = N // ncores
    WPC = (TPC + 127) // 128
    NPAD = ((N + 127) // 128) * 128
    nslabs = (NPAD + SLAB - 1) // SLAB
    NSW = (WPC + SW - 1) // SW

    si = np.asarray(si, dtype=np.int64)
    ti = np.asarray(ti, dtype=np.int64)
    core = ti // TPC
    tloc = ti - core * TPC
    w_of = tloc >> 7
    s_of = si >> 15
    ctl = tloc & 127

    counts = np.zeros((ncores, WPC, nslabs), dtype=np.int64)
    np.add.at(counts, (core, w_of, s_of), 1)
    NT = np.ceil(counts.max(axis=0) / 128).astype(np.int64)  # [WPC, nslabs]

    # --- shared emission structure ---
    buckets = []            # (w, s, t0b, nt) global tile offset + length
    tiles_w = [[] for _ in range(WPC)]   # global tile ids per window
    gcalls = []             # per g: list of (s, t0c, ct)
    sw_ranges = []          # per g: (t0, Tg)
    sw_windows = []         # per g: member window ids
    t = 0
    for g in range(NSW):
        ws = [g * SW + i for i in range(SW) if g * SW + i < WPC]
        sw_windows.append(ws)
        t0 = t
        calls = []
        for s in range(nslabs):
            c0 = t
            for w in ws:
                nt = int(NT[w, s])
                if nt == 0:
                    continue
                buckets.append((w, s, t, nt))
                tiles_w[w].extend(range(t, t + nt))
                t += nt
            k = c0
            while k < t:
                ct = min(MAX_CT, t - k)
                calls.append((s, k, ct))
                k += ct
        gcalls.append(calls)
        sw_ranges.append((t0, t - t0))
    T_total = t
    NB = max(len(buckets), 1)
    TGMAX = max((r[1] for r in sw_ranges), default=1)
    REGMAX = int(max([b[3] for b in buckets], default=1)) * 128

    # --- per-core data ---
    per_core = []
    for c in range(ncores):
        m = core == c
        csi = si[m]
        cw, cs, cctl = w_of[m], s_of[m], ctl[m]
        order = np.lexsort((cctl, cs, cw))
        csi, cw, cs, cctl = (a[order] for a in (csi, cw, cs, cctl))
        keys = cw * nslabs + cs
        starts = np.searchsorted(keys, np.arange(WPC * nslabs))
        ends = np.searchsorted(keys, np.arange(WPC * nslabs), side="right")

        si_loc = np.zeros(T_total * 128, dtype=np.int64)
        tl = np.full(T_total * 128, 255, dtype=np.int64)
        for bi, (w, s, t0b, nt) in enumerate(buckets):
            a, b = starts[w * nslabs + s], ends[w * nslabs + s]
            cnt = b - a
            if cnt == 0:
                continue
            slot0 = t0b * 128
            si_loc[slot0 : slot0 + cnt] = csi[a:b] - s * SLAB
            tl[slot0 : slot0 + cnt] = cctl[a:b]

        blocks = []
        for g in range(NSW):
            for (s, t0c, ct) in gcalls[g]:
                blocks.append(_wrap_idx(si_loc[t0c * 128 : (t0c + ct) * 128]))
        idx1 = np.concatenate(blocks, axis=1) if blocks else np.zeros((128, 8), np.int16)
        tlf = tl.astype(np.float32)
        tl_bf = tlf.reshape(T_total, 128).T.astype(ml_dtypes.bfloat16)
        tlrow = tlf.reshape(1, -1).astype(ml_dtypes.bfloat16)
        per_core.append((idx1, tl_bf, tlrow))

    sched = dict(TPC=TPC, WPC=WPC, NPAD=NPAD, nslabs=nslabs, NSW=NSW,
                 T_total=T_total, NB=NB, TGMAX=TGMAX, REGMAX=REGMAX,
                 buckets=buckets, tiles_w=tiles_w, gcalls=gcalls,
                 sw_ranges=sw_ranges, sw_windows=sw_windows)
    return sched, per_core


def build_nc(N, sched):
    WPC, NPAD, NSW = sched["WPC"], sched["NPAD"], sched["NSW"]
    T_total, NB = sched["T_total"], sched["NB"]
    TGMAX = sched["TGMAX"]
    nslabs = sched["nslabs"]
    assert TGMAX <= 64, TGMAX
    buckets, tiles_w = sched["buckets"], sched["tiles_w"]
    gcalls, sw_ranges, sw_windows = sched["gcalls"], sched["sw_ranges"], sched["sw_windows"]
    NT0 = NPAD // 128
    LROWS = WPC * 128

    # window id per global tile
    wof = np.zeros(T_total, dtype=np.int64)
    for w, tls in enumerate(tiles_w):
        for k in tls:
            wof[k] = w

    nc = bacc.Bacc("TRN2", target_bir_lowering=False, num_swdge_queues=NQ)
    srcT = nc.declare_dram_parameter("srcT", [128, NPAD], BF16, isOutput=False)
    trgTl = nc.declare_dram_parameter("trgTl", [128, LROWS], BF16, isOutput=False)
    wext = nc.declare_dram_parameter("wext", [128, 144], BF16, isOutput=False)
    iota = nc.declare_dram_parameter("iota", [128, 128], BF16, isOutput=False)
    idx1 = nc.declare_dram_parameter("idx1", [128, max(T_total * 8, 8)], I16, isOutput=False)
    tlp = nc.declare_dram_parameter("tl", [128, max(T_total, 1)], BF16, isOutput=False)
    tlrowp = nc.declare_dram_parameter("tlrow", [1, max(T_total * 128, 128)], BF16, isOutput=False)
    iotac = nc.declare_dram_parameter("iotac", [128, 1], F32, isOutput=False)
    outp = nc.declare_dram_parameter("out", [NSW * SW * 128, HF], F32, isOutput=True)
    tables = [nc.dram_tensor(f"table{s}", [SLAB, ROW], BF16) for s in range(nslabs)]

    qrr = [0]

    def next_q():
        q = qrr[0]
        qrr[0] = (q + 1) % NQ
        return q

    with tile.TileContext(nc) as tc:
        with tc.tile_pool(name="const", bufs=1) as cp:
            wext_sb = cp.tile([128, 144], BF16)
            nc.sync.dma_start(out=wext_sb[:], in_=wext[:, :])
            iota_sb = cp.tile([128, 128], BF16)
            nc.sync.dma_start(out=iota_sb[:], in_=iota[:, :])
            tl_sb = cp.tile([128, max(T_total, 1)], BF16)
            nc.sync.dma_start(out=tl_sb[:], in_=tlp[:, :])
            idx1_sb = cp.tile([128, max(T_total * 8, 8)], I16)
            nc.sync.dma_start(out=idx1_sb[:], in_=idx1[:, :])
            iotac_sb = cp.tile([128, 1], F32)
            nc.sync.dma_start(out=iotac_sb[:], in_=iotac[:, :])
            cbias = cp.tile([128, 1], F32)
            nc.vector.memset(cbias[:], -C_OFF)
            ones1 = cp.tile([1, 128], BF16)
            nc.vector.memset(ones1[:], 1.0)
            swall = cp.tile([128, WPC, 8], BF16)  # s_trg per local target

            # ---- P0b: core-local s_trg into SBUF ----
            with (
                tc.tile_pool(name="pbin", bufs=2) as pbin,
                tc.tile_pool(name="pbps", bufs=2, space="PSUM") as pbps,
            ):
                j = 0
                while j < WPC:
                    g = min(3, WPC - j)
                    t_t = pbin.tile([128, 128 * 3], BF16, tag="t", name=f"t{j}")
                    nc.sync.dma_start(out=t_t[:, : 128 * g], in_=trgTl[:, j * 128 : (j + g) * 128])
                    psB = pbps.tile([128, 8 * 3], F32, tag="psB", name=f"pb{j}")
                    for k in range(g):
                        nc.tensor.matmul(
                            out=psB[:, k * 8 : (k + 1) * 8],
                            lhsT=t_t[:, k * 128 : (k + 1) * 128],
                            rhs=wext_sb[:, 136:144],
                            start=True, stop=True,
                        )
                    psB_r = psB[:].rearrange("p (k c) -> p k c", c=8)
                    nc.scalar.copy(out=swall[:, j : j + g, :], in_=psB_r[:, :g, :])
                    j += g

            # ---- P0: packed node table (proj bf16 | s_src f32), 288B rows,
            # one DRAM tensor per slab so P1 gathers start after slab 0 ----
            CH = 16
            assert SLAB % (CH * 128) == 0
            with (
                tc.tile_pool(name="p0in", bufs=2) as p0in,
                tc.tile_pool(name="p0row", bufs=2) as p0row,
                tc.tile_pool(name="p0ps", bufs=4, space="PSUM") as p0ps,
            ):
                j = 0
                while j < NT0:
                    ch = min(CH, NT0 - j)
                    j0 = j * 128
                    s_t = p0in.tile([128, 128 * CH], BF16, tag="s", name=f"s{j}")
                    nc.sync.dma_start(out=s_t[:, : 128 * ch], in_=srcT[:, j0 : j0 + 128 * ch])
                    row = p0row.tile([128, CH, RWR], BF16, tag="row", name=f"r{j}")
                    row_f32 = row[:].bitcast(F32)  # [128, CH, 72]
                    k0 = 0
                    while k0 < ch:
                        g = min(3, ch - k0)
                        psA = p0ps.tile([128, 136 * 3], F32, tag="psA", name=f"pa{j}_{k0}")
                        for k in range(g):
                            nc.tensor.matmul(
                                out=psA[:, k * 136 : (k + 1) * 136],
                                lhsT=s_t[:, (k0 + k) * 128 : (k0 + k + 1) * 128],
                                rhs=wext_sb[:, 0:136],
                                start=True, stop=True,
                            )
                        psA_r = psA[:].rearrange("p (k c) -> p k c", c=136)
                        nc.scalar.copy(out=row[:, k0 : k0 + g, 0:HF], in_=psA_r[:, :g, 0:HF])
                        nc.scalar.copy(
                            out=row_f32[:, k0 : k0 + g, 64:72], in_=psA_r[:, :g, 128:136]
                        )
                        k0 += g
                    s = j0 // SLAB
                    jl = j0 - s * SLAB
                    nc.sync.dma_start(
                        out=tables[s][jl : jl + 128 * ch, 0:RWR].rearrange(
                            "(k p) c -> p k c", p=128
                        ),
                        in_=row[:, :ch, :],
                    )
                    j += ch

            # ---- P1: edge pass, one superwindow (SW windows) at a time ----
            with (
                tc.tile_pool(name="g1p", bufs=2) as g1p,
                tc.tile_pool(name="ohtp", bufs=2) as ohtp,
                tc.tile_pool(name="tlrp", bufs=2) as tlrp,
                tc.tile_pool(name="tlpsp", bufs=2, space="PSUM") as tlpsp,
                tc.tile_pool(name="psep", bufs=1, space="PSUM") as psep,
                tc.tile_pool(name="scp", bufs=2) as scp,
                tc.tile_pool(name="whp", bufs=2) as whp,
                tc.tile_pool(name="ohp", bufs=2) as ohp,
                tc.tile_pool(name="pswp", bufs=3, space="PSUM") as pswp,
                tc.tile_pool(name="epi", bufs=2) as epi,
            ):
                c1off = 0
                for g in range(NSW):
                    t0, Tg = sw_ranges[g]
                    ws = sw_windows[g]
                    outt = epi.tile([128, SW, HF], F32, tag="outt", name=f"o{g}")
                    if Tg > 0:
                        G1 = g1p.tile([128, TGMAX, ROW], BF16, tag="g1", name=f"g1_{g}")
                        for (s, t0c, ct) in gcalls[g]:
                            nc.gpsimd.dma_gather(
                                G1[:, t0c - t0 : t0c - t0 + ct, :],
                                tables[s][:, :],
                                idx1_sb[:, c1off : c1off + ct * 8],
                                ct * 128, ct * 128, ROW,
                                queue_num=next_q(),
                            )
                            c1off += ct * 8
                        # edge->target one-hot (transposed): PE broadcasts the
                        # tl row across partitions (ones-matmul into PSUM),
                        # then one is_equal against the partition index.
                        tlr = tlrp.tile([1, TGMAX * 128], BF16, tag="tlr", name=f"tr{g}")
                        nc.sync.dma_start(
                            out=tlr[0:1, 0 : Tg * 128],
                            in_=tlrowp[0:1, t0 * 128 : (t0 + Tg) * 128],
                        )
                        ohT = ohtp.tile([128, TGMAX * 128], BF16, tag="ohT", name=f"oT{g}")
                        c0 = 0
                        while c0 < Tg * 128:
                            cc = min(1024, Tg * 128 - c0)
                            tlps = tlpsp.tile([128, 1024], F32, tag="tlps", name=f"tp{g}_{c0}")
                            d0 = 0
                            while d0 < cc:
                                dd = min(512, cc - d0)
                                nc.tensor.matmul(
                                    out=tlps[:, d0 : d0 + dd],
                                    lhsT=ones1[0:1, :],
                                    rhs=tlr[0:1, c0 + d0 : c0 + d0 + dd],
                                    start=True, stop=True,
                                )
                                d0 += dd
                            nc.vector.tensor_scalar(
                                out=ohT[:, c0 : c0 + cc], in0=tlps[:, :cc],
                                scalar1=iotac_sb[:, 0:1], scalar2=None,
                                op0=mybir.AluOpType.is_equal,
                            )
                            c0 += cc
                        pse = psep.tile([128, TGMAX, 8], F32, tag="pse", name=f"pe{g}")
                        for kk in range(t0, t0 + Tg):
                            nc.tensor.matmul(
                                out=pse[:, kk - t0, :],
                                lhsT=ohT[:, (kk - t0) * 128 : (kk - t0 + 1) * 128],
                                rhs=swall[:, int(wof[kk]), :],
                                start=True, stop=True,
                            )
                        pse_sb = scp.tile([128, TGMAX, 8], F32, tag="pss", name=f"pb{g}")
                        nc.scalar.copy(out=pse_sb[:, :Tg, :], in_=pse[:, :Tg, :])
                        G1f = G1[:].bitcast(F32)  # [128, TGMAX, 128]
                        sc = scp.tile([128, TGMAX, 8], F32, tag="sc", name=f"sc{g}")
                        nc.vector.tensor_tensor(
                            out=sc[:, :Tg, :], in0=G1f[:, :Tg, 64:72],
                            in1=pse_sb[:, :Tg, :], op=mybir.AluOpType.add,
                        )
                        # exp(leakyrelu(s) - C) = max(exp(s - C), exp(0.2*s - C))
                        e1 = scp.tile([128, TGMAX, 8], F32, tag="e1", name=f"e1_{g}")
                        nc.scalar.activation(
                            e1[:, :Tg, :], sc[:, :Tg, :],
                            mybir.ActivationFunctionType.Exp, bias=cbias[:, 0:1],
                        )
                        e2 = scp.tile([128, TGMAX, 8], F32, tag="e2", name=f"e2_{g}")
                        nc.scalar.activation(
                            e2[:, :Tg, :], sc[:, :Tg, :],
                            mybir.ActivationFunctionType.Exp, bias=cbias[:, 0:1],
                            scale=NEG_SLOPE,
                        )
                        e_sb = scp.tile([128, TGMAX, 8], BF16, tag="e", name=f"e{g}")
                        nc.vector.tensor_tensor(
                            out=e_sb[:, :Tg, :], in0=e1[:, :Tg, :],
                            in1=e2[:, :Tg, :], op=mybir.AluOpType.max,
                        )
                        wt = whp.tile([128, TGMAX, 136], BF16, tag="wt", name=f"wt{g}")
                        nc.scalar.copy(out=wt[:, :Tg, 128:136], in_=e_sb[:, :Tg, :])
                        e_b = e_sb[:, :Tg, :].rearrange(
                            "p w (h o) -> p w h o", o=1
                        ).to_broadcast([128, Tg, 8, 16])
                        nc.vector.tensor_tensor(
                            out=wt[:, :Tg, 0:128].rearrange("p w (h f) -> p w h f", f=16),
                            in0=G1[:, :Tg, 0:128].rearrange("p w (h f) -> p w h f", f=16),
                            in1=e_b, op=mybir.AluOpType.mult,
                        )
                        oh = ohp.tile([128, TGMAX, 128], BF16, tag="oh", name=f"oh{g}")
                        iota_b = iota_sb[:].rearrange("p (o c) -> p o c", o=1).to_broadcast(
                            [128, Tg, 128]
                        )
                        tl_b = tl_sb[:, t0 : t0 + Tg].rearrange(
                            "p (w o) -> p w o", o=1
                        ).to_broadcast([128, Tg, 128])
                        nc.vector.tensor_tensor(
                            out=oh[:, :Tg, :], in0=iota_b, in1=tl_b,
                            op=mybir.AluOpType.is_equal,
                        )
                    for wi, w in enumerate(ws):
                        tls = tiles_w[w]
                        if not tls:
                            nc.vector.memset(outt[:, wi, :], 0.0)
                            continue
                        psw = pswp.tile([128, 136], F32, tag="ps", name=f"ps{g}_{wi}")
                        for i, kk in enumerate(tls):
                            nc.tensor.matmul(
                                out=psw[:], lhsT=oh[:, kk - t0, :], rhs=wt[:, kk - t0, :],
                                start=(i == 0), stop=(i == len(tls) - 1),
                            )
                        dn = epi.tile([128, 8], F32, tag="dn", name=f"dn{g}_{wi}")
                        nc.vector.tensor_scalar_add(out=dn[:], in0=psw[:, 128:136], scalar1=1e-16)
                        rc = epi.tile([128, 8], F32, tag="rc", name=f"rc{g}_{wi}")
                        nc.vector.reciprocal(out=rc[:], in_=dn[:])
                        rc_b = rc[:].rearrange("p (h o) -> p h o", o=1).to_broadcast([128, 8, 16])
                        nc.vector.tensor_tensor(
                            out=outt[:, wi, :].rearrange("p (h f) -> p h f", f=16),
                            in0=psw[:, 0:HF].rearrange("p (h f) -> p h f", f=16),
                            in1=rc_b, op=mybir.AluOpType.mult,
                        )
                    nc.sync.dma_start(
                        out=outp[g * SW * 128 : (g * SW + len(ws)) * 128, :].rearrange(
                            "(k p) c -> p k c", p=128
                        ),
                        in_=outt[:, : len(ws), :],
                    )
    nc.compile()
    return nc


def host_prep(trg, src, W_trg, W_src, a_src, a_trg, N, ncores, TPC, WPC, NPAD):
    LROWS = WPC * 128
    src2 = np.asarray(src, dtype=np.float32).reshape(-1, D)[:N]
    trg2 = np.asarray(trg, dtype=np.float32).reshape(-1, D)[:N]
    W_src = np.asarray(W_src, dtype=np.float32)
    W_trg = np.asarray(W_trg, dtype=np.float32)
    a_src = np.asarray(a_src, dtype=np.float32)
    a_trg = np.asarray(a_trg, dtype=np.float32)
    w_s = np.einsum("hf,hfd->hd", a_src, W_src.reshape(NH, FOUT, D))
    w_t = np.einsum("hf,hfd->hd", a_trg, W_trg.reshape(NH, FOUT, D))
    wext = np.zeros((128, 144), dtype=np.float32)
    wext[:, 0:HF] = W_src.T
    wext[:, HF : HF + 8] = w_s.T
    wext[:, 136:144] = w_t.T
    bf = ml_dtypes.bfloat16
    srcT = np.zeros((128, NPAD), dtype=np.float32)
    srcT[:, :N] = src2.T
    trgTls = []
    for c in range(ncores):
        t = np.zeros((128, LROWS), dtype=np.float32)
        t[:, :TPC] = trg2[c * TPC : (c + 1) * TPC].T
        trgTls.append(t.astype(bf))
    iota = np.tile(np.arange(128, dtype=np.float32), (128, 1))
    return srcT.astype(bf), trgTls, wext.astype(bf), iota.astype(bf)


_CACHE = {}


def run_graph(trg, src, edge_index, W_trg, W_src, a_src, a_trg, N, ncores,
              trace=False):
    global LAST_EXEC_NS
    si = np.asarray(edge_index[0], dtype=np.int64)
    ti = np.asarray(edge_index[1], dtype=np.int64)
    sched, per_core = build_schedule(si, ti, N, ncores)
    TPC, WPC, NPAD = sched["TPC"], sched["WPC"], sched["NPAD"]
    T_total = sched["T_total"]

    srcT, trgTls, wext, iota = host_prep(
        trg, src, W_trg, W_src, a_src, a_trg, N, ncores, TPC, WPC, NPAD
    )

    key = (N, ncores, T_total, tuple(int(b[2]) for b in sched["buckets"]))
    if key not in _CACHE:
        _CACHE[key] = build_nc(N, sched)
    nc = _CACHE[key]

    iotac = np.arange(128, dtype=np.float32).reshape(128, 1)
    in_maps = []
    for c in range(ncores):
        idx1, tl_bf, tlrow = per_core[c]
        i1 = np.zeros((128, max(T_total * 8, 8)), dtype=np.int16)
        i1[:, : idx1.shape[1]] = idx1
        tlz = np.full((128, max(T_total, 1)), 255.0, dtype=ml_dtypes.bfloat16)
        tlz[:, : tl_bf.shape[1]] = tl_bf
        trz = np.full((1, max(T_total * 128, 128)), 255.0, dtype=ml_dtypes.bfloat16)
        trz[:, : tlrow.shape[1]] = tlrow
        in_maps.append(
            {"srcT": srcT, "trgTl": trgTls[c], "wext": wext, "iota": iota,
             "idx1": i1, "tl": tlz, "tlrow": trz, "iotac": iotac}
        )

    if trace:
        trace = _install_trace_shim()
    res = run_bass_kernel_spmd(nc, in_maps, core_ids=list(range(ncores)), trace=trace)
    LAST_EXEC_NS = res.exec_time_ns
    out = np.zeros((N, HF), dtype=np.float32)
    for c in range(ncores):
        out[c * TPC : (c + 1) * TPC] = res.results[c]["out"][:TPC]
    return out


def kernel(trg, src, edge_index, W_trg, W_src, a_src, a_trg):
    N = 100000
    out = run_graph(trg, src, edge_index, W_trg, W_src, a_src, a_trg, N, 8,
                    trace=bool(os.environ.get("KERNEL_TRACE")))
    return out.reshape(1, N, HF)
